# revision 15
# baseline (speedup 1.0000x reference)
"""Trainium2 Bass kernel for nn_DTKSA (sparse top-k channel attention).

Self-contained: kernel(**inputs) takes the FULL float32 inputs (as produced by
the oracle's setup_inputs) and returns the FULL float32 output, running an SPMD
Bass/Tile kernel on 8 NeuronCores.

Sharding: spatial row-bands everywhere. Each core owns 24 full-res rows
(12 pooled rows) plus a 2-full-res-row halo for the depthwise conv. The
channel attention (48x48 per head) contracts over the FULL spatial extent, so
each core computes a per-head Gram partial over its spatial slice and ONE
fp16 AllReduce (~38KB) sums them; diag norms ride in the same payload. The
softmax coefficient matrix is computed redundantly on every core; CW @ v,
gelu and the 1x1 proj are spatially local. The nearest-neighbor 2x upsample
commutes with the 1x1 proj, so the kernel emits the pooled-res projected
output (fp16) and the host replicates it to full res.

Perf structure (v2):
- 16 PE warmup matmuls on the identity right after its DMA pull the HAM
  clock gate to 8/8 before the first real matmul.
- qkv conv (stage A) runs in m-block pairs so pooled blocks finish early;
  each (m,k) matmul is split into two concurrent 64-channel col-tiles
  (halves the exposed LDWEIGHTS cost).
- the 2x2 max pool is two tensor_tensor max ops (col-pair from PSUM, then
  row-pair in fp16) instead of a 1x-mode tensor_reduce: ~435ns vs ~670ns.
- the 3x3 depthwise conv packs 2 taps per matmul: partition-stacked shifted
  copies of the pooled block (built by SBUF-SBUF DMA while the next A pair
  computes) let one K=128 matmul apply 2 taps to 64 channels, and the two
  64-channel halves run as concurrent col-tiles. 9 taps = 4 pair rounds + 1
  single round instead of 9 full rounds.
- the Gram AllReduce runs in fp16 and a tiny warmup AllReduce at kernel
  start absorbs the cc-stream setup latency.
- ACT table sets are prefetched (sqrt during the v phase, exp behind the
  D-phase sort chain) so no table load sits on the critical path.
"""

import threading

import numpy as np

import bass_rust
import concourse.bass as bass
import concourse.mybir as mybir
import concourse.tile as tile
from concourse.vector_clock import ScopedClock
from concourse.bass_utils import run_bass_kernel_spmd

# ----------------------------------------------------------------------------
# Tile tail-drain workaround: this walrus build rejects >1 sync-wait on the
# SP/CTRL Drain that TileContext emits at exit. Keep one wait on the drain and
# give each extra wait its own follow-up Drain (1-wait CTRL drains are what
# all_engine_barrier itself emits, so they are known-good).
# ----------------------------------------------------------------------------


def _patched_drain_and_barrier(self, tick_clock, wait_clock):
    nc = self.nc
    drain_inst = nc.sync.drain()
    wait_clock.add_sem_waits(
        drain_inst.ins, ScopedClock({None: tick_clock.global_clock})
    )
    si = drain_inst.ins.sync_info
    conds = list(si.on_wait or []) if si is not None else []
    if len(conds) > 1:
        si.on_wait = conds[:1]
        for cond in conds[1:]:
            extra = nc.sync.drain()
            extra.ins.sync_info = bass_rust.SyncInfo(on_wait=[cond], on_update=[])
    nc.all_engine_barrier()
    assert self.sems is not None
    popped = nc._tile_sem_poison_stack.pop()
    assert popped is self._sem_poison
    nc.clear_and_free_semaphores(list(self.sems.allocated().values()))
    nc.all_engine_barrier()


tile.TileContext._drain_and_barrier = _patched_drain_and_barrier

# This walrus build also caps the number of sync-wait commands a single
# instruction may carry (Tile can attach more). Split any excess waits onto
# same-engine NOPs inserted immediately before the instruction.
_WAIT_CAP = 1


def _split_sync_waits(nc, cap=_WAIT_CAP):
    for fn in nc.m.functions:
        for blk in fn.blocks:
            insts = list(blk.instructions)
            out, n_added = [], 0
            for ins in insts:
                si = getattr(ins, "sync_info", None)
                waits = list(si.on_wait or []) if si is not None else []
                if len(waits) > cap:
                    si.on_wait = waits[:cap]
                    rest = waits[cap:]
                    for i in range(0, len(rest), cap):
                        nop = mybir.InstNoOp(
                            name=f"{ins.name}_w{i}",
                            engine=ins.engine,
                            ins=[], outs=[],
                            sync_info=bass_rust.SyncInfo(
                                on_wait=rest[i:i + cap], on_update=[]),
                        )
                        out.append(nop)
                        n_added += 1
                out.append(ins)
            if n_added:
                blk.instructions = out

# ----------------------------------------------------------------------------
# Problem constants (hardcoded per the harness contract).
# ----------------------------------------------------------------------------
NC = 8                  # cores
DIM = 384               # channels
HEADS = 8
C = DIM // HEADS        # 48 channels/head
HF, WF = 192, 192       # full-res spatial
HP, WP = 96, 96         # pooled spatial
DS = 2
RPC = HF // NC          # 24 full-res rows per core
PRC = HP // NC          # 12 pooled rows per core
XR = RPC + 4            # 28 full-res rows incl. 2-row halo each side
NSP = XR * WF           # 5376 spatial columns in stage-1
NT = 384                # stage-1 matmul N-tile == 2 full-res rows (5376=14*384)
PR = PRC + 2            # 14 pooled rows incl. 1 halo row each side
PW = WP + 2             # 98: pooled row stride with zero pad cols
MB = DIM * 3 // 128     # 9 channel blocks of 128 in qkv
TAPS = [(dy, dx) for dy in (-1, 0, 1) for dx in (-1, 0, 1)]
# tap pairs packed into one K=128 matmul: (t, t+3) differ by +1 pooled row
# (+98 elements); (6,7) differ by +1 col. tap 8 runs alone at full width.
PAIRS = [(0, 3), (1, 4), (2, 5), (6, 7)]
PAIR_TAG = ["s98", "s98", "s98", "s1"]
PAIR_R0 = [(0, 0), (0, 1), (0, 2), (2, 0)]   # (row off from 4s, col off)
KKS = [C // 2, C * 2 // 3, C * 3 // 4, C * 4 // 5]   # 24, 32, 36, 38
NEG = -1.0e30
CCG = 48 * 384          # gram region of the collective payload
CCN = CCG + 768         # + sumsq tail

F32 = mybir.dt.float32
F16 = mybir.dt.float16
AX = mybir.AxisListType
ALU = mybir.AluOpType
ACTF = mybir.ActivationFunctionType

N_WARM_MM = 40          # PE warmup matmuls at kernel start (~4.3us busy)


def build_kernel():
    nc = bass.Bass(target_bir_lowering=False, debug=False)

    x_in = nc.declare_dram_parameter("x_slice", [3, 4, 128, 1344], F16, isOutput=False)
    wqkv_in = nc.declare_dram_parameter("wqkvT", [3, 128, 1152], F16, isOutput=False)
    dwp_in = nc.declare_dram_parameter("dwp", [128, 9, 2, 4, 64], F16, isOutput=False)
    dw8_in = nc.declare_dram_parameter("dw8", [128, 9, 128], F16, isOutput=False)
    wproj_in = nc.declare_dram_parameter("wprojT", [4, 96, 384], F16, isOutput=False)
    bqkv_in = nc.declare_dram_parameter("bqkv", [128, 9, 3], F32, isOutput=False)
    bdw_in = nc.declare_dram_parameter("bdw", [128, 9], F32, isOutput=False)
    bproj_in = nc.declare_dram_parameter("bproj", [128, 3], F32, isOutput=False)
    ident_in = nc.declare_dram_parameter("ident", [128, 128], F16, isOutput=False)
    temp_in = nc.declare_dram_parameter("tempg", [96, 4], F32, isOutput=False)
    maskk_in = nc.declare_dram_parameter("maskk", [16, 8], F32, isOutput=False)
    atile_in = nc.declare_dram_parameter("atile", [96, 16], F32, isOutput=False)
    y_out = nc.declare_dram_parameter("y_slice", [3, 128, PRC * WP], F16, isOutput=True)

    with tile.TileContext(nc) as tc:
        with (
            tc.tile_pool(name="persist", bufs=1) as persist,
            tc.tile_pool(name="dram", bufs=1, space="DRAM") as dram,
        ):
            # ---- persistent SBUF tiles -----------------------------------
            pooled = [persist.tile([128, PR * PW], F16, tag=f"pooled{m}", name=f"pooled{m}")
                      for m in range(MB)]
            ident = persist.tile([128, 128], F16, tag="ident")
            dwp = persist.tile([128, 9, 2, 4, 64], F16, tag="dwp")
            dw8 = persist.tile([128, 9, 128], F16, tag="dw8")
            bqkv = persist.tile([128, 9, 3], F32, tag="bqkv")
            bdw = persist.tile([128, 9], F32, tag="bdw")
            bproj = persist.tile([128, 3], F32, tag="bproj")
            tempg = persist.tile([96, 4], F32, tag="tempg")
            atile = persist.tile([96, 16], F32, tag="atile")
            ones16 = persist.tile([16, 48], F32, tag="ones16")
            maskk = persist.tile([16, 4, 2], F32, tag="maskk")
            x_sb = [persist.tile([128, NSP], F16, tag=f"x{k}", name=f"x{k}")
                    for k in range(3)]
            wq = persist.tile([128, 3, 1152], F16, tag="wq")
            qk_buf = [persist.tile([128, PRC * WP], F16, tag=f"qk{m}",
                                   name=f"qk{m}") for m in range(6)]
            qkT = [persist.tile([128, 768], F16, tag=f"qkT{j}",
                                name=f"qkT{j}") for j in range(9)]
            v_pair = [persist.tile([96, PRC * WP], F16, tag=f"vp{p}",
                                   name=f"vp{p}") for p in range(4)]
            y_pair = [persist.tile([96, PRC * WP], F16, tag=f"yp{p}",
                                   name=f"yp{p}") for p in range(4)]
            g_buf = persist.tile([48, 400], F16, tag="gbuf")
            wproj = persist.tile([96, 4, 384], F16, tag="wproj")
            identf = persist.tile([96, 96], F32, tag="identf")
            sq_acc = persist.tile([128, 6], F32, tag="sq_acc")
            sq16 = persist.tile([128, 6], F16, tag="sq16")
            sq_scr = persist.tile([128, PRC * WP], F16, tag="sq_scr")
            scr1 = persist.tile([1, 4], F32, tag="scr1")

            # warmup AllReduce: absorbs the cc-stream setup cost so the real
            # Gram AllReduce starts fast. Input copied from a real parameter
            # so nothing reads uninitialized DRAM.
            ccw_in = dram.tile([16], F32, name="ccw_in")
            ccw_out = dram.tile([16], F32, name="ccw_out", addr_space="Shared")
            nc.sync.dma_start(ccw_in[0:16],
                              temp_in.rearrange("p f -> (p f)")[0:16])
            nc.gpsimd.collective_compute(
                "AllReduce", ALU.add,
                replica_groups=[list(range(NC))],
                ins=[ccw_in.opt()], outs=[ccw_out.opt()],
            )

            # DMA issue order matters (~650ns issue cost per descriptor on a
            # queue): the sync queue carries ident (warmup mms), the m=0/1
            # qkv weights and the x quarters that feed the first matmuls;
            # the small tables ride the scalar engine's DGE queue so the
            # first pool evictions (which need bqkv) are never blocked
            # behind the x stream.
            nc.sync.dma_start(ident[:], ident_in[:, :])
            for k in range(3):
                nc.sync.dma_start(wq[:, k, 0:256], wqkv_in[k, :, 0:256])
            for q in range(2):
                for k in range(3):
                    nc.sync.dma_start(
                        x_sb[k][:, q * 1344:(q + 1) * 1344], x_in[k, q, :, :])
            for k in range(3):
                nc.sync.dma_start(wq[:, k, 256:1152], wqkv_in[k, :, 256:1152])
            for q in range(2, 4):
                for k in range(3):
                    nc.sync.dma_start(
                        x_sb[k][:, q * 1344:(q + 1) * 1344], x_in[k, q, :, :])
            nc.sync.dma_start(dwp[:], dwp_in[:, :, :, :, :])
            nc.sync.dma_start(dw8[:], dw8_in[:, :, :])
            nc.sync.dma_start(wproj[:], wproj_in.rearrange("k p f -> p k f"))

            nc.scalar.dma_start(bqkv[:], bqkv_in[:, :, :])
            nc.scalar.dma_start(bdw[:], bdw_in[:, :])
            nc.scalar.dma_start(bproj[:], bproj_in[:, :])
            nc.scalar.dma_start(tempg[:], temp_in[:, :])
            nc.scalar.dma_start(atile[:], atile_in[:, :])
            nc.scalar.dma_start(
                maskk.rearrange("p g two -> p (g two)"), maskk_in[:, :])

            nc.vector.tensor_copy(identf[:], ident[0:96, 0:96])
            nc.vector.memset(ones16[:], 1.0)
            nc.vector.memset(scr1[:], 1.0)
            # zero only the pad columns of the pooled buffers; every row of
            # cols 1..96 is written by the pool reduce
            pl3s = [pooled[m].rearrange("p (r c) -> p r c", c=PW)
                    for m in range(MB)]
            for m in range(MB):
                nc.vector.memset(pl3s[m][:, :, 0], 0.0)
                nc.vector.memset(pl3s[m][:, :, 97], 0.0)

            ccg_in = dram.tile([CCN], F16, name="ccg_in")
            ccg_out = dram.tile([CCN], F16, name="ccg_out",
                                addr_space="Shared")

            # ---- PE warmup: pull HAM to 8/8 while the x DMA streams ------
            with tc.tile_pool(name="ps_wu", bufs=1, space="PSUM") as ps_wu:
                wups = ps_wu.tile([128, 128], F32, tag="wu", name="wu")
                for _ in range(N_WARM_MM):
                    nc.tensor.matmul(wups, lhsT=ident[:], rhs=ident[:],
                                     start=True, stop=True)

            # ======== Phase A: qkv 1x1 conv + fused 2x2 max pool ==========
            # Each (m,k) is two concurrent 64-channel col-tile matmuls. The
            # PSUM tile is evicted by ACT with the qkv bias fused in (bias
            # then max == max then bias for a per-channel constant; the halo
            # rows get their edge-masked bias column), then the 2x2 pool is
            # two fp16 SBUF tensor_tensor max ops on DVE.
            def a_tile(m, nt, ps_pool, cm_pool):
                ps = ps_pool.tile([128, NT], F32, tag="ps_a", name="ps_a")
                c0 = nt * NT
                for k in range(3):
                    for h in range(2):
                        # per-col-tile start/stop: start=True clears
                        # has_written only for this matmul's col groups
                        nc.tensor.matmul(
                            ps[h * 64:(h + 1) * 64, :],
                            lhsT=wq[:, k, m * 128 + h * 64:m * 128 + (h + 1) * 64],
                            rhs=x_sb[k][:, c0:c0 + NT],
                            start=(k == 0),
                            stop=(k == 2),
                            skip_group_check=True,
                        )
                bcol = 0 if 1 <= nt <= 12 else (1 if nt == 0 else 2)
                cm = cm_pool.tile([128, NT], F16, tag="cm", name="cm")
                nc.scalar.activation(
                    cm[:], ps[:], ACTF.Identity,
                    bias=bqkv[:, m, bcol:bcol + 1], scale=1.0)
                cm4 = cm.rearrange("p (r c two) -> p r c two", c=WP, two=2)
                cmq = cm_pool.tile([128, 2, WP], F16, tag="cmq", name="cmq")
                nc.vector.tensor_tensor(
                    cmq[:], cm4[:, :, :, 0], cm4[:, :, :, 1], op=ALU.max)
                nc.vector.tensor_tensor(
                    pl3s[m][:, nt, 1:97], cmq[:, 0, :], cmq[:, 1, :], op=ALU.max)

            # stacked shifted copies for the 2-tap depthwise rounds: for each
            # 64-channel half, partitions 64..127 hold the same data shifted
            # +98 elements (next pooled row) or +1 element (next col).
            def stack_tiles(m, stk_pool):
                tiles = {}
                for h in range(2):
                    for rel, tag in ((98, "s98"), (1, "s1")):
                        t = stk_pool.tile([128, PR * PW], F16,
                                          tag=f"{tag}h{h}",
                                          name=f"{tag}h{h}_{m}")
                        src = pooled[m]
                        nc.sync.dma_start(
                            t[0:64, :], src[h * 64:h * 64 + 64, :])
                        nc.sync.dma_start(
                            t[64:128, 0:PR * PW - rel],
                            src[h * 64:h * 64 + 64, rel:PR * PW])
                        tiles[(tag, h)] = t
                return tiles

            # ======== Phase B: depthwise 3x3 as 4 pair rounds + 1 =========
            def dw_block(m, stks, ph_b, ps_b):
                pl3 = pl3s[m]
                pss = [ps_b.tile([128, 4 * WP], F32, tag=f"ps_b{s}",
                                 name=f"ps_b{s}_{m}") for s in range(3)]
                for r in range(4):
                    dyof, c0 = PAIR_R0[r]
                    tag = PAIR_TAG[r]
                    for h in range(2):
                        lhs = dwp[:, m, h, r, :]
                        stk3 = stks[(tag, h)].rearrange(
                            "p (r c) -> p r c", c=PW)
                        for s in range(3):
                            # per-col-tile start: each half clears its own
                            # col groups' has_written on its first round
                            nc.tensor.matmul(
                                pss[s][h * 64:(h + 1) * 64, :],
                                lhsT=lhs,
                                rhs=stk3[:, 4 * s + dyof:4 * s + dyof + 4,
                                         c0:c0 + 96],
                                start=(r == 0),
                                stop=False,
                                skip_group_check=True,
                            )
                for s in range(3):
                    # tap 8 accumulates full-width onto both halves
                    nc.tensor.matmul(
                        pss[s], lhsT=dw8[:, m, :],
                        rhs=pl3[:, 4 * s + 2:4 * s + 6, 2:98],
                        start=False, stop=True,
                        skip_group_check=True,
                    )
                if m < 6:
                    for s in range(3):
                        nc.scalar.activation(
                            qk_buf[m][:, s * 384:(s + 1) * 384],
                            pss[s][:], ACTF.Identity,
                            bias=bdw[:, m:m + 1], scale=1.0)
                    # local sum-of-squares (free-dim accum on ACT)
                    nc.scalar.activation(
                        sq_scr[:], qk_buf[m][:], ACTF.Square,
                        accum_out=sq_acc[:, m:m + 1])
                else:
                    # v: evict + b_dw, DMA-rearrange into head pairs
                    vs = ph_b.tile([128, PRC * WP], F16, tag="vstage")
                    for s in range(3):
                        nc.scalar.activation(
                            vs[:, s * 384:(s + 1) * 384], pss[s][:],
                            ACTF.Identity, bias=bdw[:, m:m + 1], scale=1.0)
                    base = (m - 6) * 128
                    lo_pair, lo_off = divmod(base, 96)
                    take0 = 96 - lo_off if lo_off else 96
                    nc.sync.dma_start(
                        v_pair[lo_pair][lo_off:lo_off + take0, :],
                        vs[0:take0, :])
                    if take0 < 128:
                        nc.sync.dma_start(
                            v_pair[lo_pair + 1][0:128 - take0, :],
                            vs[take0:128, :])

            # transposes feed the Gram: qkT[jj][sp, ch]
            def qk_transposes(m, ps_t, last=False):
                for jj in range(9):
                    tp = ps_t.tile([128, 128], F16, tag="tp", name="tp")
                    nc.tensor.transpose(
                        tp[:], qk_buf[m][:, jj * 128:(jj + 1) * 128],
                        ident[:])
                    # spread the PSUM evictions over both ACT and DVE; the
                    # final block goes to DVE (ACT is busy with dw evicts)
                    eng = nc.vector.tensor_copy if (last or jj % 2) \
                        else nc.scalar.copy
                    eng(qkT[jj][:, m * 128:(m + 1) * 128], tp[:])

            # qk pipeline: A for pair p, then stacks(p), then dw+transposes
            # for pair p-1 (stack DMAs hide under the next pair's A compute)
            apairs = [(0, 1), (2, 3), (4, 5)]
            with (
                tc.tile_pool(name="stk", bufs=4) as stk_pool,
            ):
                stks = {}
                with (
                    tc.tile_pool(name="ps_a1", bufs=3, space="PSUM") as ps_a1,
                    tc.tile_pool(name="cm1", bufs=4) as cm1,
                    tc.tile_pool(name="ph_b1", bufs=2) as ph_b1,
                    tc.tile_pool(name="ps_b1", bufs=1, space="PSUM") as ps_b1,
                    tc.tile_pool(name="ps_t", bufs=2, space="PSUM") as ps_t,
                ):
                    for pi, pair in enumerate(apairs):
                        for nt in range(14):
                            for m in pair:
                                a_tile(m, nt, ps_a1, cm1)
                        for m in pair:
                            stks[m] = stack_tiles(m, stk_pool)
                        if pi > 0:
                            for m in apairs[pi - 1]:
                                dw_block(m, stks.pop(m), ph_b1, ps_b1)
                            for m in apairs[pi - 1]:
                                qk_transposes(m, ps_t)
                    for m in apairs[2]:
                        dw_block(m, stks.pop(m), ph_b1, ps_b1)
                    nc.vector.tensor_copy(sq16[:], sq_acc[:])
                    nc.sync.dma_start(
                        ccg_in[CCG:CCN].rearrange("(b p) -> p b", p=128),
                        sq16[:])
                    # prefetch the sqrt ACT table set: reading sq16 makes
                    # Tile schedule this after the last dw Square (whose set
                    # would otherwise evict sqrt's), well before phase D
                    nc.scalar.sqrt(scr1[0:1, 2:3], sq16[0:1, 0:1])
                    for m in apairs[2]:
                        qk_transposes(m, ps_t, last=(m == 5))

                # ======== Phase C: Gram, AllReduce ========================
                with tc.tile_pool(name="ps_g", bufs=1, space="PSUM") as ps_g:
                    gp = [ps_g.tile([48, 48], F32, tag=f"gp{h}", name=f"gp{h}")
                          for h in range(HEADS)]
                    for jj in range(9):
                        for h in range(HEADS):
                            nc.tensor.matmul(
                                gp[h],
                                lhsT=qkT[jj][:, h * 48:(h + 1) * 48],
                                rhs=qkT[jj][:, 384 + h * 48:384 + (h + 1) * 48],
                                start=(jj == 0), stop=(jj == 8))
                    # DVE evicts: ACT is still draining dw evictions and the
                    # A2 pool below cannot reuse these banks until all 8 are
                    # read out (~150ns each on DVE vs ~580ns on ACT)
                    for h in range(HEADS):
                        nc.vector.tensor_copy(
                            g_buf[:, h * 48:(h + 1) * 48], gp[h][:])

                    nc.sync.dma_start(
                        ccg_in[0:CCG].rearrange("(p f) -> p f", p=48),
                        g_buf[:, 0:384])
                    nc.gpsimd.collective_compute(
                        "AllReduce", ALU.add,
                        replica_groups=[list(range(NC))],
                        ins=[ccg_in.opt()], outs=[ccg_out.opt()],
                    )

                # ======== Phase A/B for v (overlaps the AllReduce) ========
                with (
                    tc.tile_pool(name="ps_a2", bufs=6, space="PSUM") as ps_a2,
                    tc.tile_pool(name="cm2", bufs=4) as cm2,
                    tc.tile_pool(name="ps_w2", bufs=1, space="PSUM") as ps_w2,
                ):
                    # bridge the gram->AR->A2 transition so HAM stays warm
                    wps = ps_w2.tile([128, 384], F32, tag="warm2", name="warm2")
                    for _ in range(2):
                        nc.tensor.matmul(wps, lhsT=wq[:, 0, 0:128],
                                         rhs=x_sb[0][:, 0:384],
                                         start=True, stop=True)
                    for m in range(6, MB):
                        for nt in range(14):
                            a_tile(m, nt, ps_a2, cm2)
                        stks[m] = stack_tiles(m, stk_pool)
                with (
                    tc.tile_pool(name="ph_b2", bufs=2) as ph_b2,
                    tc.tile_pool(name="ps_b2", bufs=1, space="PSUM") as ps_b2,
                ):
                    for m in range(6, MB):
                        dw_block(m, stks.pop(m), ph_b2, ps_b2)

            # collective result (norms region; the Gram region feeds a_all
            # directly from ccg_out)
            nc.sync.dma_start(
                g_buf[:, 384:400],
                ccg_out[CCG:CCN].rearrange("(col i) -> i col", i=48))

            # ======== Phase D: attention coefficient matrices =============
            with (
                tc.tile_pool(name="ph_d", bufs=1) as ph_d,
                tc.tile_pool(name="ps_d", bufs=1, space="PSUM") as ps_d,
            ):
                # norms: sumsq in g_buf[:, 384:400] (48, 16):
                # col h = ||q_i||^2 head h, col 8+h = ||k_i||^2 head h
                sumsq = ph_d.tile([48, 16], F32, tag="sumsq")
                nc.vector.tensor_scalar_max(
                    sumsq[:], g_buf[:, 384:400], 1.0e-12)
                nrm = ph_d.tile([48, 16], F32, tag="nrm")
                nc.scalar.sqrt(nrm[:], sumsq[:])
                # prefetch the exp table set right after the norm sqrt (the
                # nrm input pins the schedule); the load hides behind the
                # DVE-side norm broadcast and the first sort chain
                nc.scalar.activation(scr1[0:1, 3:4], nrm[0:1, 0:1], ACTF.Exp)
                rns = ph_d.tile([48, 16], F32, tag="rns")
                nc.vector.reciprocal(rns[:], nrm[:])

                # transpose rns -> (16, 48): rows = q/k x head, cols = channel
                rtp = ps_d.tile([48, 48], F32, tag="rtp")
                nc.tensor.transpose(rtp[0:16, :], rns[:],
                                    identf[0:48, 0:48])
                rnsT = ph_d.tile([16, 48], F32, tag="rnsT")
                nc.scalar.copy(rnsT[:], rtp[0:16, 0:48])

                # broadcast k-norms along partitions: per half, mask-select
                # the 4 needed rows of rnsT into group-blocks and matmul with
                # an all-ones stationary (out[p, (g,c)] = rk[2g+half, c])
                rkb = ph_d.tile([96, 192], F32, tag="rkb")
                rk_stage = ph_d.tile([48, 192], F32, tag="rk_stage")
                for half in range(2):
                    rhs3 = ph_d.tile([16, 4, 48], F32, tag="rhs3",
                                     name=f"rhs3{half}")
                    nc.vector.tensor_tensor(
                        rhs3[:],
                        rnsT[:, None, :].to_broadcast([16, 4, 48]),
                        maskk[:, :, half, None].to_broadcast([16, 4, 48]),
                        op=ALU.mult)
                    rkps = ps_d.tile([48, 192], F32, tag="rkps",
                                     name=f"rkps{half}")
                    nc.tensor.matmul(
                        rkps, lhsT=ones16[:],
                        rhs=rhs3.rearrange("p g d -> p (g d)"),
                        start=True, stop=True)
                    nc.scalar.copy(rkb[0:48, :] if half == 0 else rk_stage[:],
                                   rkps[:])
                nc.sync.dma_start(rkb[48:96, :], rk_stage[:])
                # q-norms, partition-aligned: rqb (96, 4); temperature folds
                # into the q side (A = rq*temp * G * rk)
                rqb = ph_d.tile([96, 4], F32, tag="rqb")
                rns2 = rns.rearrange("p (g x) -> p g x", x=2)
                nc.sync.dma_start(rqb[0:48, :], rns2[:, 0:4, 0])
                nc.sync.dma_start(rqb[48:96, :], rns2[:, 0:4, 1])
                nc.vector.tensor_tensor(rqb[:], rqb[:], tempg[:], op=ALU.mult)

                # A packed (96, 4*48): group g = heads (2g | 2g+1), loaded
                # straight from the AllReduce output in DRAM (fp16), scaled
                # into fp32 in aw
                a_all = ph_d.tile([96, 192], F16, tag="a_all")
                aw = ph_d.tile([96, 192], F32, tag="aw")
                g_v = ccg_out[0:CCG].rearrange(
                    "(c g two d) -> c g two d", g=4, two=2, d=48)
                for half in range(2):
                    nc.sync.dma_start(
                        a_all[half * 48:half * 48 + 48, :]
                        .rearrange("c (g d) -> c g d", d=48),
                        g_v[:, :, half, :])
                a3 = a_all.rearrange("p (g c) -> p g c", c=48)
                aw3 = aw.rearrange("p (g c) -> p g c", c=48)
                nc.vector.tensor_tensor(
                    aw3, a3,
                    rqb[:, :, None].to_broadcast([96, 4, 48]),
                    op=ALU.mult)
                nc.vector.tensor_tensor(aw[:], aw[:], rkb[:], op=ALU.mult)

                # Per-group pipeline: top-40 selection -> coefficients ->
                # CW -> block-diag transpose -> CW @ v + gelu. Group g's
                # PE/ACT tail overlaps group g+1's DVE sort chain.
                srt = ph_d.tile([96, 4, 40], F32, tag="sorted")
                scr = ph_d.tile([96, 192], F32, tag="scratch")
                es = ph_d.tile([96, 4, 40], F32, tag="esort")
                nrow = ph_d.tile([96, 4], F32, tag="nrow")
                sall = ph_d.tile([96, 4, 4], F32, tag="sall")
                call = ph_d.tile([96, 4, 4], F32, tag="call")
                msum = ph_d.tile([96, 192], F32, tag="msum")
                mb_t = ph_d.tile([96, 192], F32, tag="mb")
                cw = ph_d.tile([96, 192], F32, tag="cw")
                cwh = ph_d.tile([96, 192], F16, tag="cwh")
                cwt_l = [ph_d.tile([96, 96], F16, tag=f"cwt{g}",
                                   name=f"cwt{g}") for g in range(4)]
                at4 = atile.rearrange("p (g b) -> p g b", b=4)

                with (
                    tc.tile_pool(name="ph_e", bufs=2) as ph_e,
                    tc.tile_pool(name="ps_e", bufs=2, space="PSUM") as ps_e,
                    tc.tile_pool(name="ps_w", bufs=1, space="PSUM") as ps_w,
                ):
                    def pe_warm(src_ap, nm):
                        # dependency-spread dummy work: keeps the PE's HAM
                        # activity window open through the DVE-serial stretch
                        # so the tail matmuls run at full clock. The first
                        # transpose anchors the timing to the D-chain; the
                        # matmuls add enough duty cycle to register as busy.
                        warm = ps_w.tile([48, 512], F32, tag="warm", name=nm)
                        nc.tensor.transpose(warm[:, 0:96], src_ap, identf[:])
                        for ww in range(5):
                            nc.tensor.matmul(
                                warm[:, 0:512], lhsT=ident[:, 0:48],
                                rhs=x_sb[0][:, 0:512],
                                start=True, stop=True)

                    for g in range(4):
                        src = aw[:, g * 48:(g + 1) * 48]
                        dst = scr[:, g * 48:(g + 1) * 48]
                        # top-40 per row via 5 rounds of max8 + match_replace
                        # (the final round needs no replace)
                        for r in range(5):
                            nc.vector.max(srt[:, g, r * 8:(r + 1) * 8],
                                          src if r == 0 else dst)
                            if r < 4:
                                nc.vector.match_replace(
                                    out=dst,
                                    in_to_replace=srt[:, g, r * 8:(r + 1) * 8],
                                    in_values=src if r == 0 else dst,
                                    imm_value=NEG)
                        # prefix sums of exp(sorted - rowmax): fused exp +
                        # free-dim accumulate on ACT (bias = -rowmax)
                        nc.scalar.mul(nrow[:, g:g + 1], srt[:, g, 0:1], -1.0)
                        nc.scalar.activation(
                            es[:, g, 0:KKS[0]], srt[:, g, 0:KKS[0]],
                            ACTF.Exp, bias=nrow[:, g:g + 1], scale=1.0,
                            accum_out=sall[:, g, 0:1])
                        for bb in range(1, 4):
                            nc.scalar.activation(
                                es[:, g, KKS[bb - 1]:KKS[bb]],
                                srt[:, g, KKS[bb - 1]:KKS[bb]],
                                ACTF.Exp, bias=nrow[:, g:g + 1], scale=1.0,
                                accum_out=sall[:, g, bb:bb + 1])
                            nc.vector.tensor_add(
                                sall[:, g, bb:bb + 1], sall[:, g, bb:bb + 1],
                                sall[:, g, bb - 1:bb])
                        nc.vector.reciprocal(call[:, g, :], sall[:, g, :])
                        nc.vector.tensor_tensor(
                            call[:, g, :], call[:, g, :], at4[:, g, :],
                            op=ALU.mult)
                        pe_warm(scr[:, g * 48:(g + 1) * 48], f"warm_a{g}")
                        # msum = sum_b c_b*[A >= t_b]; CW = exp(A-rowmax)*msum
                        # fused per branch: (A is_ge t_b) * c_b in one op
                        for bb in range(4):
                            tgt = (msum if bb == 0 else mb_t)[:, g * 48:
                                                              (g + 1) * 48]
                            nc.vector.tensor_scalar(
                                tgt, src,
                                srt[:, g, KKS[bb] - 1:KKS[bb]],
                                call[:, g, bb:bb + 1],
                                op0=ALU.is_ge, op1=ALU.mult)
                            if bb > 0:
                                nc.vector.tensor_add(
                                    msum[:, g * 48:(g + 1) * 48],
                                    msum[:, g * 48:(g + 1) * 48], tgt)
                        pe_warm(msum[:, g * 48:(g + 1) * 48], f"warm_b{g}")
                        cwg = cw[:, g * 48:(g + 1) * 48]
                        nc.scalar.activation(
                            cwg, src, ACTF.Exp, bias=nrow[:, g:g + 1],
                            scale=1.0)
                        nc.vector.tensor_tensor(
                            cwh[:, g * 48:(g + 1) * 48], cwg,
                            msum[:, g * 48:(g + 1) * 48], op=ALU.mult)
                        # block-diag CW -> transpose -> cwt; CW @ v + gelu
                        # immediately so the PE fills during the next
                        # group's sort chain
                        bd = ph_e.tile([96, 96], F16, tag="bdiag")
                        nc.vector.memset(bd[:], 0.0)
                        nc.vector.tensor_copy(
                            bd[0:48, 0:48], cwh[0:48, g * 48:(g + 1) * 48])
                        nc.sync.dma_start(
                            bd[48:96, 48:96], cwh[48:96, g * 48:(g + 1) * 48])
                        tps = ps_d.tile([96, 96], F16, tag="tps")
                        nc.tensor.transpose(tps[:], bd[:], ident[0:96, 0:96])
                        nc.vector.tensor_copy(cwt_l[g][:], tps[:])
                        for s in range(3):
                            ops = ps_e.tile([96, 384], F32, tag="ops",
                                            name=f"ops{g}_{s}")
                            nc.tensor.matmul(
                                ops, lhsT=cwt_l[g][:],
                                rhs=v_pair[g][:, s * 384:(s + 1) * 384],
                                start=True, stop=True)
                            nc.scalar.activation(
                                y_pair[g][:, s * 384:(s + 1) * 384], ops[:],
                                ACTF.Gelu)

                # ==== Phase F: 1x1 proj at pooled res =====================
                with (
                    tc.tile_pool(name="ps_f", bufs=3, space="PSUM") as ps_f,
                    tc.tile_pool(name="ph_f", bufs=2) as ph_f,
                ):
                    for m in range(3):
                        pj = ph_f.tile([128, PRC * WP], F16, tag="pj",
                                       name=f"pj{m}")
                        for s in range(3):
                            psf = ps_f.tile([128, 384], F32, tag="ps_f",
                                            name=f"psf{s}_{m}")
                            for k in range(4):
                                nc.tensor.matmul(
                                    psf,
                                    lhsT=wproj[:, k, m * 128:(m + 1) * 128],
                                    rhs=y_pair[k][:, s * 384:(s + 1) * 384],
                                    start=(k == 0),
                                    stop=(k == 3),
                                )
                            # bias evict on DVE: ACT is busy with the gelus
                            nc.vector.tensor_scalar(
                                pj[:, s * 384:(s + 1) * 384], psf[:],
                                bproj[:, m:m + 1], None, op0=ALU.add)
                        nc.sync.dma_start(y_out[m, :, :], pj[:])

    _split_sync_waits(nc)
    return nc


# ----------------------------------------------------------------------------
# Host-side input preparation / sharding / gather
# ----------------------------------------------------------------------------

def _prep_core_inputs(x, w_qkv, b_qkv, w_dw, b_dw, w_proj, b_proj,
                      temperature, a1, a2, a3, a4):
    x = np.asarray(x, np.float32).reshape(DIM, HF, WF)
    w_qkv = np.asarray(w_qkv, np.float32)
    w_dw = np.asarray(w_dw, np.float32).reshape(3 * DIM, 3, 3)
    w_proj = np.asarray(w_proj, np.float32)

    wqkvT = np.ascontiguousarray(w_qkv.T).reshape(3, 128, 3 * DIM).astype(np.float16)
    wprojT = np.ascontiguousarray(w_proj.T).reshape(4, 96, DIM).astype(np.float16)

    # packed depthwise weights: dwp[p, m, h, r, c] applies tap PAIRS[r][p//64]
    # to channel m*128+h*64+c (diagonal in c = p%64); dw8 is the tap-8 diag.
    dwp = np.zeros((128, MB, 2, 4, 64), np.float32)
    pa = np.arange(128)
    ca = pa % 64
    for m in range(MB):
        for h in range(2):
            for r, (t0, t1) in enumerate(PAIRS):
                taps = np.where(pa < 64, t0, t1)
                dy = np.array([TAPS[t][0] for t in taps]) + 1
                dx = np.array([TAPS[t][1] for t in taps]) + 1
                dwp[pa, m, h, r, ca] = w_dw[m * 128 + h * 64 + ca, dy, dx]
    dw8 = np.zeros((128, MB, 128), np.float32)
    for m in range(MB):
        dw8[np.arange(128), m, np.arange(128)] = w_dw[m * 128 + np.arange(128), 2, 2]

    bq = np.asarray(b_qkv, np.float32).reshape(MB, 128)
    bd = np.asarray(b_dw, np.float32).reshape(MB, 128)
    bp = np.asarray(b_proj, np.float32).reshape(3, 128)

    ident = np.eye(128, dtype=np.float16)
    t8 = np.asarray(temperature, np.float32).reshape(HEADS)
    tempg = np.empty((96, 4), np.float32)
    for g in range(4):
        tempg[0:48, g] = t8[2 * g]
        tempg[48:96, g] = t8[2 * g + 1]
    maskk_h = np.zeros((16, 4, 2), np.float32)
    for g in range(4):
        for half in range(2):
            maskk_h[8 + 2 * g + half, g, half] = 1.0
    maskk_h = maskk_h.reshape(16, 8)
    avec = np.array([np.float32(a1[0]), np.float32(a2[0]),
                     np.float32(a3[0]), np.float32(a4[0])], np.float32)
    atile = np.tile(avec, (96, 4)).astype(np.float32)

    # x: pad 2 halo rows of zeros top/bottom, slice per core, cast fp16
    xp = np.zeros((DIM, HF + 4, WF), np.float16)
    xp[:, 2:HF + 2, :] = x
    in_maps = []
    for c in range(NC):
        xs = xp[:, c * RPC:c * RPC + XR, :]                  # (384, 28, 192)
        xs = xs.reshape(3, 128, XR * WF).reshape(3, 128, 4, 1344)
        xs = np.ascontiguousarray(xs.transpose(0, 2, 1, 3))
        bqkv3 = np.stack([
            bq.T, bq.T * (1.0 if c > 0 else 0.0),
            bq.T * (1.0 if c < NC - 1 else 0.0)], axis=2)     # (128, 9, 3)
        in_maps.append({
            "x_slice": xs,
            "wqkvT": wqkvT,
            "dwp": dwp.astype(np.float16),
            "dw8": dw8.astype(np.float16),
            "wprojT": wprojT,
            "bqkv": np.ascontiguousarray(bqkv3, np.float32),
            "bdw": np.ascontiguousarray(bd.T),
            "bproj": np.ascontiguousarray(bp.T),
            "ident": ident,
            "tempg": tempg,
            "maskk": maskk_h,
            "atile": atile,
        })
    return in_maps


_CACHE = {}
_CACHE_LOCK = threading.Lock()


def _make_runner():
    """Compile once; return a callable in_maps -> list[{name: array}].

    Mirrors concourse.bass2jax.run_bass_via_pjrt but caches the jitted
    executable so repeat kernel() calls do not recompile.
    """
    import jax
    import concourse.mybir as mybir
    from concourse import bass2jax
    from jax.experimental.shard_map import shard_map
    from jax.sharding import Mesh, PartitionSpec

    nc = build_kernel()
    bass2jax.install_neuronx_cc_hook()
    partition_name = (nc.partition_id_tensor.name
                      if nc.partition_id_tensor else None)
    in_names, out_names, out_avals, zero_outs = [], [], [], []
    for alloc in nc.m.functions[0].allocations:
        if not isinstance(alloc, mybir.MemoryLocationSet):
            continue
        name = alloc.memorylocations[0].name
        if alloc.kind == "ExternalInput":
            if name != partition_name:
                in_names.append(name)
        elif alloc.kind == "ExternalOutput":
            shape = tuple(alloc.tensor_shape)
            dtype = mybir.dt.np(alloc.dtype)
            out_names.append(name)
            out_avals.append(jax.core.ShapedArray(shape, dtype))
            zero_outs.append(np.zeros(shape, dtype))
    n_params = len(in_names)
    n_outs = len(out_avals)
    all_names = list(in_names) + list(out_names)
    if partition_name is not None:
        all_names.append(partition_name)
    donate = tuple(range(n_params, n_params + n_outs))

    def _body(*args):
        operands = list(args)
        if partition_name is not None:
            operands.append(bass2jax.partition_id_tensor())
        return tuple(bass2jax._bass_exec_p.bind(
            *operands,
            out_avals=tuple(out_avals),
            in_names=tuple(all_names),
            out_names=tuple(out_names),
            lowering_input_output_aliases=(),
            sim_require_finite=True,
            sim_require_nnan=True,
            nc=nc,
        ))

    devices = jax.devices()[:NC]
    mesh = Mesh(np.asarray(devices), ("core",))
    in_specs = (PartitionSpec("core"),) * (n_params + n_outs)
    out_specs = (PartitionSpec("core"),) * n_outs
    sharded = jax.jit(
        shard_map(_body, mesh=mesh, in_specs=in_specs, out_specs=out_specs,
                  check_rep=False),
        donate_argnums=donate, keep_unused=True)

    import jax.numpy as jnp
    sharding = jax.sharding.NamedSharding(mesh, PartitionSpec("core"))
    zeros_dev = jax.jit(
        lambda: tuple(
            jnp.zeros((NC * z.shape[0], *z.shape[1:]), z.dtype)
            for z in zero_outs),
        out_shardings=tuple(sharding for _ in zero_outs))

    def upload(in_maps):
        concat_in = [
            np.concatenate([np.asarray(in_maps[c][nm]) for c in range(NC)],
                           axis=0)
            for nm in in_names[:n_params]
        ]
        return [jax.device_put(a, sharding) for a in concat_in]

    def execute(dev_args):
        out_arrs = sharded(*dev_args, *zeros_dev())
        jax.block_until_ready(out_arrs)
        return out_arrs

    def run(in_maps):
        out_arrs = execute(upload(in_maps))
        return [
            {nm: np.asarray(out_arrs[i]).reshape(NC, *out_avals[i].shape)[c]
             for i, nm in enumerate(out_names)}
            for c in range(NC)
        ]

    run.upload = upload
    run.execute = execute
    return run


def kernel(**inputs) -> np.ndarray:
    with _CACHE_LOCK:
        runner = _CACHE.get("runner")
        if runner is None:
            runner = _make_runner()
            _CACHE["runner"] = runner
    in_maps = _prep_core_inputs(**inputs)
    results = runner(in_maps)
    out = np.empty((1, DIM, HF, WF), np.float32)
    for c in range(NC):
        y = results[c]["y_slice"].astype(np.float32).reshape(DIM, PRC, WP)
        up = np.repeat(np.repeat(y, DS, axis=1), DS, axis=2)
        out[0, :, c * RPC:(c + 1) * RPC, :] = up
    return out


# revision 18
# speedup vs baseline: 1.0718x; 1.0718x over previous
"""Trainium2 Bass kernel for nn_DTKSA (sparse top-k channel attention).

Self-contained: kernel(**inputs) takes the FULL float32 inputs (as produced by
the oracle's setup_inputs) and returns the FULL float32 output, running an SPMD
Bass/Tile kernel on 8 NeuronCores.

Sharding: spatial row-bands everywhere. Each core owns 24 full-res rows
(12 pooled rows) plus a 2-full-res-row halo for the depthwise conv. The
channel attention (48x48 per head) contracts over the FULL spatial extent, so
each core computes a per-head Gram partial over its spatial slice and ONE
fp16 AllReduce (~38KB) sums them; diag norms ride in the same payload. The
softmax coefficient matrix is computed redundantly on every core; CW @ v,
gelu and the 1x1 proj are spatially local. The nearest-neighbor 2x upsample
commutes with the 1x1 proj, so the kernel emits the pooled-res projected
output (fp16) and the host replicates it to full res.

Perf structure (v2):
- 16 PE warmup matmuls on the identity right after its DMA pull the HAM
  clock gate to 8/8 before the first real matmul.
- qkv conv (stage A) runs in m-block pairs so pooled blocks finish early;
  each (m,k) matmul is split into two concurrent 64-channel col-tiles
  (halves the exposed LDWEIGHTS cost).
- the 2x2 max pool is two tensor_tensor max ops (col-pair from PSUM, then
  row-pair in fp16) instead of a 1x-mode tensor_reduce: ~435ns vs ~670ns.
- the 3x3 depthwise conv packs 2 taps per matmul: partition-stacked shifted
  copies of the pooled block (built by SBUF-SBUF DMA while the next A pair
  computes) let one K=128 matmul apply 2 taps to 64 channels, and the two
  64-channel halves run as concurrent col-tiles. 9 taps = 4 pair rounds + 1
  single round instead of 9 full rounds.
- the Gram AllReduce runs in fp16 and a tiny warmup AllReduce at kernel
  start absorbs the cc-stream setup latency.
- ACT table sets are prefetched (sqrt during the v phase, exp behind the
  D-phase sort chain) so no table load sits on the critical path.
"""

import threading

import numpy as np

import bass_rust
import concourse.bass as bass
import concourse.mybir as mybir
import concourse.tile as tile
from concourse.vector_clock import ScopedClock
from concourse.bass_utils import run_bass_kernel_spmd

# ----------------------------------------------------------------------------
# Tile tail-drain workaround: this walrus build rejects >1 sync-wait on the
# SP/CTRL Drain that TileContext emits at exit. Keep one wait on the drain and
# give each extra wait its own follow-up Drain (1-wait CTRL drains are what
# all_engine_barrier itself emits, so they are known-good).
# ----------------------------------------------------------------------------


def _patched_drain_and_barrier(self, tick_clock, wait_clock):
    nc = self.nc
    drain_inst = nc.sync.drain()
    wait_clock.add_sem_waits(
        drain_inst.ins, ScopedClock({None: tick_clock.global_clock})
    )
    si = drain_inst.ins.sync_info
    conds = list(si.on_wait or []) if si is not None else []
    if len(conds) > 1:
        si.on_wait = conds[:1]
        for cond in conds[1:]:
            extra = nc.sync.drain()
            extra.ins.sync_info = bass_rust.SyncInfo(on_wait=[cond], on_update=[])
    nc.all_engine_barrier()
    assert self.sems is not None
    popped = nc._tile_sem_poison_stack.pop()
    assert popped is self._sem_poison
    nc.clear_and_free_semaphores(list(self.sems.allocated().values()))
    nc.all_engine_barrier()


tile.TileContext._drain_and_barrier = _patched_drain_and_barrier

# This walrus build also caps the number of sync-wait commands a single
# instruction may carry (Tile can attach more). Split any excess waits onto
# same-engine NOPs inserted immediately before the instruction.
_WAIT_CAP = 1


def _split_sync_waits(nc, cap=_WAIT_CAP):
    for fn in nc.m.functions:
        for blk in fn.blocks:
            insts = list(blk.instructions)
            out, n_added = [], 0
            for ins in insts:
                si = getattr(ins, "sync_info", None)
                waits = list(si.on_wait or []) if si is not None else []
                if len(waits) > cap:
                    si.on_wait = waits[:cap]
                    rest = waits[cap:]
                    for i in range(0, len(rest), cap):
                        nop = mybir.InstNoOp(
                            name=f"{ins.name}_w{i}",
                            engine=ins.engine,
                            ins=[], outs=[],
                            sync_info=bass_rust.SyncInfo(
                                on_wait=rest[i:i + cap], on_update=[]),
                        )
                        out.append(nop)
                        n_added += 1
                out.append(ins)
            if n_added:
                blk.instructions = out

# ----------------------------------------------------------------------------
# Problem constants (hardcoded per the harness contract).
# ----------------------------------------------------------------------------
NC = 8                  # cores
DIM = 384               # channels
HEADS = 8
C = DIM // HEADS        # 48 channels/head
HF, WF = 192, 192       # full-res spatial
HP, WP = 96, 96         # pooled spatial
DS = 2
RPC = HF // NC          # 24 full-res rows per core
PRC = HP // NC          # 12 pooled rows per core
XR = RPC + 4            # 28 full-res rows incl. 2-row halo each side
NSP = XR * WF           # 5376 spatial columns in stage-1
NT = 384                # stage-1 matmul N-tile == 2 full-res rows (5376=14*384)
PR = PRC + 2            # 14 pooled rows incl. 1 halo row each side
PW = WP + 2             # 98: pooled row stride with zero pad cols
MB = DIM * 3 // 128     # 9 channel blocks of 128 in qkv
TAPS = [(dy, dx) for dy in (-1, 0, 1) for dx in (-1, 0, 1)]
# tap pairs packed into one K=128 matmul: (t, t+3) differ by +1 pooled row
# (+98 elements); (6,7) differ by +1 col. tap 8 runs alone at full width.
PAIRS = [(0, 3), (1, 4), (2, 5), (6, 7)]
PAIR_TAG = ["s98", "s98", "s98", "s1"]
PAIR_R0 = [(0, 0), (0, 1), (0, 2), (2, 0)]   # (row off from 4s, col off)
KKS = [C // 2, C * 2 // 3, C * 3 // 4, C * 4 // 5]   # 24, 32, 36, 38
NEG = -1.0e30
CCG = 48 * 384          # gram region of the collective payload
CCN = CCG + 768         # + sumsq tail

F32 = mybir.dt.float32
F16 = mybir.dt.float16
AX = mybir.AxisListType
ALU = mybir.AluOpType
ACTF = mybir.ActivationFunctionType

N_WARM_MM = 40          # PE warmup matmuls at kernel start (~4.3us busy)


def build_kernel():
    nc = bass.Bass(target_bir_lowering=False, debug=False)

    x_in = nc.declare_dram_parameter("x_slice", [3, 4, 128, 1344], F16, isOutput=False)
    wqkv_in = nc.declare_dram_parameter("wqkvT", [3, 128, 1152], F16, isOutput=False)
    dwp_in = nc.declare_dram_parameter("dwp", [128, 9, 2, 4, 64], F16, isOutput=False)
    dw8_in = nc.declare_dram_parameter("dw8", [128, 9, 128], F16, isOutput=False)
    wproj_in = nc.declare_dram_parameter("wprojT", [4, 96, 384], F16, isOutput=False)
    bqkv_in = nc.declare_dram_parameter("bqkv", [128, 9, 3], F32, isOutput=False)
    bdw_in = nc.declare_dram_parameter("bdw", [128, 9], F32, isOutput=False)
    bproj_in = nc.declare_dram_parameter("bproj", [128, 3], F32, isOutput=False)
    ident_in = nc.declare_dram_parameter("ident", [128, 128], F16, isOutput=False)
    temp_in = nc.declare_dram_parameter("tempg", [96, 4], F32, isOutput=False)
    maskk_in = nc.declare_dram_parameter("maskk", [16, 8], F32, isOutput=False)
    atile_in = nc.declare_dram_parameter("atile", [96, 16], F32, isOutput=False)
    y_out = nc.declare_dram_parameter("y_slice", [3, 128, PRC * WP], F16, isOutput=True)

    with tile.TileContext(nc) as tc:
        with (
            tc.tile_pool(name="persist", bufs=1) as persist,
            tc.tile_pool(name="dram", bufs=1, space="DRAM") as dram,
        ):
            # ---- persistent SBUF tiles -----------------------------------
            pooled = [persist.tile([128, PR * PW], F16, tag=f"pooled{m}", name=f"pooled{m}")
                      for m in range(MB)]
            ident = persist.tile([128, 128], F16, tag="ident")
            dwp = persist.tile([128, 9, 2, 4, 64], F16, tag="dwp")
            dw8 = persist.tile([128, 9, 128], F16, tag="dw8")
            bqkv = persist.tile([128, 9, 3], F32, tag="bqkv")
            bdw = persist.tile([128, 9], F32, tag="bdw")
            bproj = persist.tile([128, 3], F32, tag="bproj")
            tempg = persist.tile([96, 4], F32, tag="tempg")
            atile = persist.tile([96, 16], F32, tag="atile")
            ones16 = persist.tile([16, 48], F32, tag="ones16")
            maskk = persist.tile([16, 4, 2], F32, tag="maskk")
            x_sb = [persist.tile([128, NSP], F16, tag=f"x{k}", name=f"x{k}")
                    for k in range(3)]
            wq = persist.tile([128, 3, 1152], F16, tag="wq")
            qk_buf = [persist.tile([128, PRC * WP], F16, tag=f"qk{m}",
                                   name=f"qk{m}") for m in range(6)]
            qkT = [persist.tile([128, 768], F16, tag=f"qkT{j}",
                                name=f"qkT{j}") for j in range(9)]
            v_pair = [persist.tile([96, PRC * WP], F16, tag=f"vp{p}",
                                   name=f"vp{p}") for p in range(4)]
            y_pair = [persist.tile([96, PRC * WP], F16, tag=f"yp{p}",
                                   name=f"yp{p}") for p in range(4)]
            g_buf = persist.tile([48, 400], F16, tag="gbuf")
            wproj = persist.tile([96, 4, 384], F16, tag="wproj")
            identf = persist.tile([96, 96], F32, tag="identf")
            sq_acc = persist.tile([128, 6], F32, tag="sq_acc")
            sq16 = persist.tile([128, 6], F16, tag="sq16")
            sq_scr = persist.tile([128, PRC * WP], F16, tag="sq_scr")
            scr1 = persist.tile([1, 4], F32, tag="scr1")

            # warmup AllReduce: absorbs the cc-stream setup cost so the real
            # Gram AllReduce starts fast. Input copied from a real parameter
            # so nothing reads uninitialized DRAM.
            ccw_in = dram.tile([16], F32, name="ccw_in")
            ccw_out = dram.tile([16], F32, name="ccw_out", addr_space="Shared")
            nc.sync.dma_start(ccw_in[0:16],
                              temp_in.rearrange("p f -> (p f)")[0:16])
            nc.gpsimd.collective_compute(
                "AllReduce", ALU.add,
                replica_groups=[list(range(NC))],
                ins=[ccw_in.opt()], outs=[ccw_out.opt()],
            )

            # DMA issue order matters (~650ns issue cost per descriptor on a
            # queue): the sync queue carries ident (warmup mms), the m=0/1
            # qkv weights and the x quarters that feed the first matmuls;
            # the small tables ride the scalar engine's DGE queue so the
            # first pool evictions (which need bqkv) are never blocked
            # behind the x stream.
            nc.sync.dma_start(ident[:], ident_in[:, :])
            for k in range(3):
                nc.sync.dma_start(wq[:, k, 0:256], wqkv_in[k, :, 0:256])
            for q in range(2):
                for k in range(3):
                    nc.sync.dma_start(
                        x_sb[k][:, q * 1344:(q + 1) * 1344], x_in[k, q, :, :])
            for k in range(3):
                nc.sync.dma_start(wq[:, k, 256:1152], wqkv_in[k, :, 256:1152])
            for q in range(2, 4):
                for k in range(3):
                    nc.sync.dma_start(
                        x_sb[k][:, q * 1344:(q + 1) * 1344], x_in[k, q, :, :])
            nc.sync.dma_start(dwp[:], dwp_in[:, :, :, :, :])
            nc.sync.dma_start(dw8[:], dw8_in[:, :, :])
            nc.sync.dma_start(wproj[:], wproj_in.rearrange("k p f -> p k f"))

            nc.scalar.dma_start(bqkv[:], bqkv_in[:, :, :])
            nc.scalar.dma_start(bdw[:], bdw_in[:, :])
            nc.scalar.dma_start(bproj[:], bproj_in[:, :])
            nc.scalar.dma_start(tempg[:], temp_in[:, :])
            nc.scalar.dma_start(atile[:], atile_in[:, :])
            nc.scalar.dma_start(
                maskk.rearrange("p g two -> p (g two)"), maskk_in[:, :])

            nc.vector.tensor_copy(identf[:], ident[0:96, 0:96])
            nc.vector.memset(ones16[:], 1.0)
            nc.vector.memset(scr1[:], 1.0)
            # zero only the pad columns of the pooled buffers; every row of
            # cols 1..96 is written by the pool reduce
            pl3s = [pooled[m].rearrange("p (r c) -> p r c", c=PW)
                    for m in range(MB)]
            for m in range(MB):
                nc.vector.memset(pl3s[m][:, :, 0], 0.0)
                nc.vector.memset(pl3s[m][:, :, 97], 0.0)

            ccg_in = dram.tile([CCN], F16, name="ccg_in")
            ccg_out = dram.tile([CCN], F16, name="ccg_out",
                                addr_space="Shared")

            # ---- PE warmup: pull HAM to 8/8 while the x DMA streams ------
            with tc.tile_pool(name="ps_wu", bufs=1, space="PSUM") as ps_wu:
                wups = ps_wu.tile([128, 128], F32, tag="wu", name="wu")
                for _ in range(N_WARM_MM):
                    nc.tensor.matmul(wups, lhsT=ident[:], rhs=ident[:],
                                     start=True, stop=True)

            # ======== Phase A: qkv 1x1 conv + fused 2x2 max pool ==========
            # Each (m,k) is two concurrent 64-channel col-tile matmuls. The
            # PSUM tile is evicted by ACT with the qkv bias fused in (bias
            # then max == max then bias for a per-channel constant; the halo
            # rows get their edge-masked bias column), then the 2x2 pool is
            # two fp16 SBUF tensor_tensor max ops on DVE.
            def a_tile(m, nt, ps_pool, cm_pool):
                ps = ps_pool.tile([128, NT], F32, tag="ps_a", name="ps_a")
                c0 = nt * NT
                for k in range(3):
                    for h in range(2):
                        # per-col-tile start/stop: start=True clears
                        # has_written only for this matmul's col groups
                        nc.tensor.matmul(
                            ps[h * 64:(h + 1) * 64, :],
                            lhsT=wq[:, k, m * 128 + h * 64:m * 128 + (h + 1) * 64],
                            rhs=x_sb[k][:, c0:c0 + NT],
                            start=(k == 0),
                            stop=(k == 2),
                            skip_group_check=True,
                        )
                bcol = 0 if 1 <= nt <= 12 else (1 if nt == 0 else 2)
                cm = cm_pool.tile([128, NT], F16, tag="cm", name="cm")
                nc.scalar.activation(
                    cm[:], ps[:], ACTF.Identity,
                    bias=bqkv[:, m, bcol:bcol + 1], scale=1.0)
                cm4 = cm.rearrange("p (r c two) -> p r c two", c=WP, two=2)
                cmq = cm_pool.tile([128, 2, WP], F16, tag="cmq", name="cmq")
                nc.vector.tensor_tensor(
                    cmq[:], cm4[:, :, :, 0], cm4[:, :, :, 1], op=ALU.max)
                nc.vector.tensor_tensor(
                    pl3s[m][:, nt, 1:97], cmq[:, 0, :], cmq[:, 1, :], op=ALU.max)

            # stacked shifted copies for the 2-tap depthwise rounds: for each
            # 64-channel half, partitions 64..127 hold the same data shifted
            # +98 elements (next pooled row) or +1 element (next col).
            def stack_tiles(m, stk_pool):
                tiles = {}
                for h in range(2):
                    for rel, tag in ((98, "s98"), (1, "s1")):
                        t = stk_pool.tile([128, PR * PW], F16,
                                          tag=f"{tag}h{h}",
                                          name=f"{tag}h{h}_{m}")
                        src = pooled[m]
                        nc.sync.dma_start(
                            t[0:64, :], src[h * 64:h * 64 + 64, :])
                        nc.sync.dma_start(
                            t[64:128, 0:PR * PW - rel],
                            src[h * 64:h * 64 + 64, rel:PR * PW])
                        tiles[(tag, h)] = t
                return tiles

            # ======== Phase B: depthwise 3x3 as 4 pair rounds + 1 =========
            def dw_block(m, stks, ph_b, ps_b):
                pl3 = pl3s[m]
                pss = [ps_b.tile([128, 4 * WP], F32, tag=f"ps_b{s}",
                                 name=f"ps_b{s}_{m}") for s in range(3)]
                for r in range(4):
                    dyof, c0 = PAIR_R0[r]
                    tag = PAIR_TAG[r]
                    for h in range(2):
                        lhs = dwp[:, m, h, r, :]
                        stk3 = stks[(tag, h)].rearrange(
                            "p (r c) -> p r c", c=PW)
                        for s in range(3):
                            # per-col-tile start: each half clears its own
                            # col groups' has_written on its first round
                            nc.tensor.matmul(
                                pss[s][h * 64:(h + 1) * 64, :],
                                lhsT=lhs,
                                rhs=stk3[:, 4 * s + dyof:4 * s + dyof + 4,
                                         c0:c0 + 96],
                                start=(r == 0),
                                stop=False,
                                skip_group_check=True,
                            )
                for s in range(3):
                    # tap 8 accumulates full-width onto both halves
                    nc.tensor.matmul(
                        pss[s], lhsT=dw8[:, m, :],
                        rhs=pl3[:, 4 * s + 2:4 * s + 6, 2:98],
                        start=False, stop=True,
                        skip_group_check=True,
                    )
                if m < 6:
                    for s in range(3):
                        nc.scalar.activation(
                            qk_buf[m][:, s * 384:(s + 1) * 384],
                            pss[s][:], ACTF.Identity,
                            bias=bdw[:, m:m + 1], scale=1.0)
                    # local sum-of-squares (free-dim accum on ACT)
                    nc.scalar.activation(
                        sq_scr[:], qk_buf[m][:], ACTF.Square,
                        accum_out=sq_acc[:, m:m + 1])
                else:
                    # v: evict + b_dw, DMA-rearrange into head pairs
                    vs = ph_b.tile([128, PRC * WP], F16, tag="vstage")
                    for s in range(3):
                        nc.scalar.activation(
                            vs[:, s * 384:(s + 1) * 384], pss[s][:],
                            ACTF.Identity, bias=bdw[:, m:m + 1], scale=1.0)
                    base = (m - 6) * 128
                    lo_pair, lo_off = divmod(base, 96)
                    take0 = 96 - lo_off if lo_off else 96
                    nc.sync.dma_start(
                        v_pair[lo_pair][lo_off:lo_off + take0, :],
                        vs[0:take0, :])
                    if take0 < 128:
                        nc.sync.dma_start(
                            v_pair[lo_pair + 1][0:128 - take0, :],
                            vs[take0:128, :])

            # transposes feed the Gram: qkT[jj][sp, ch]
            def qk_transposes(m, ps_t, last=False):
                for jj in range(9):
                    tp = ps_t.tile([128, 128], F16, tag="tp", name="tp")
                    nc.tensor.transpose(
                        tp[:], qk_buf[m][:, jj * 128:(jj + 1) * 128],
                        ident[:])
                    # spread the PSUM evictions over both ACT and DVE; the
                    # final block goes to DVE (ACT is busy with dw evicts)
                    eng = nc.vector.tensor_copy if (last or jj % 2) \
                        else nc.scalar.copy
                    eng(qkT[jj][:, m * 128:(m + 1) * 128], tp[:])

            # qk pipeline: A for pair p, then stacks(p), then dw+transposes
            # for pair p-1 (stack DMAs hide under the next pair's A compute)
            apairs = [(0, 1), (2, 3), (4, 5)]
            with (
                tc.tile_pool(name="stk", bufs=4) as stk_pool,
            ):
                stks = {}
                with (
                    tc.tile_pool(name="ps_a1", bufs=3, space="PSUM") as ps_a1,
                    tc.tile_pool(name="cm1", bufs=4) as cm1,
                    tc.tile_pool(name="ph_b1", bufs=2) as ph_b1,
                    tc.tile_pool(name="ps_b1", bufs=1, space="PSUM") as ps_b1,
                    tc.tile_pool(name="ps_t", bufs=2, space="PSUM") as ps_t,
                ):
                    for pi, pair in enumerate(apairs):
                        for nt in range(14):
                            for m in pair:
                                a_tile(m, nt, ps_a1, cm1)
                        for m in pair:
                            stks[m] = stack_tiles(m, stk_pool)
                        if pi > 0:
                            for m in apairs[pi - 1]:
                                dw_block(m, stks.pop(m), ph_b1, ps_b1)
                            for m in apairs[pi - 1]:
                                qk_transposes(m, ps_t)
                    for m in apairs[2]:
                        dw_block(m, stks.pop(m), ph_b1, ps_b1)
                    nc.vector.tensor_copy(sq16[:], sq_acc[:])
                    nc.sync.dma_start(
                        ccg_in[CCG:CCN].rearrange("(b p) -> p b", p=128),
                        sq16[:])
                    # prefetch the sqrt ACT table set: reading sq16 makes
                    # Tile schedule this after the last dw Square (whose set
                    # would otherwise evict sqrt's), well before phase D
                    nc.scalar.sqrt(scr1[0:1, 2:3], sq16[0:1, 0:1])
                    for m in apairs[2]:
                        qk_transposes(m, ps_t, last=(m == 5))

                # ======== Phase C: Gram, AllReduce ========================
                with tc.tile_pool(name="ps_g", bufs=1, space="PSUM") as ps_g:
                    gp = [ps_g.tile([48, 48], F32, tag=f"gp{h}", name=f"gp{h}")
                          for h in range(HEADS)]
                    for jj in range(9):
                        for h in range(HEADS):
                            nc.tensor.matmul(
                                gp[h],
                                lhsT=qkT[jj][:, h * 48:(h + 1) * 48],
                                rhs=qkT[jj][:, 384 + h * 48:384 + (h + 1) * 48],
                                start=(jj == 0), stop=(jj == 8))
                    # ACT evicts: the A2 pool below reuses these banks, and
                    # on the DVE the evicts get queue-blocked behind split
                    # sync-wait NOPs tied to the A2 matmuls (measured ~9us
                    # priority-inversion stall); the ACT queue is clear here
                    for h in range(HEADS):
                        nc.scalar.copy(g_buf[:, h * 48:(h + 1) * 48], gp[h][:])

                    nc.sync.dma_start(
                        ccg_in[0:CCG].rearrange("(p f) -> p f", p=48),
                        g_buf[:, 0:384])
                    nc.gpsimd.collective_compute(
                        "AllReduce", ALU.add,
                        replica_groups=[list(range(NC))],
                        ins=[ccg_in.opt()], outs=[ccg_out.opt()],
                    )

                # ======== Phase A/B for v (overlaps the AllReduce) ========
                with (
                    tc.tile_pool(name="ps_a2", bufs=6, space="PSUM") as ps_a2,
                    tc.tile_pool(name="cm2", bufs=4) as cm2,
                ):
                    for m in range(6, MB):
                        for nt in range(14):
                            a_tile(m, nt, ps_a2, cm2)
                        stks[m] = stack_tiles(m, stk_pool)
                with (
                    tc.tile_pool(name="ph_b2", bufs=2) as ph_b2,
                    tc.tile_pool(name="ps_b2", bufs=1, space="PSUM") as ps_b2,
                ):
                    for m in range(6, MB):
                        dw_block(m, stks.pop(m), ph_b2, ps_b2)

            # collective result (norms region; the Gram region feeds a_all
            # directly from ccg_out)
            nc.sync.dma_start(
                g_buf[:, 384:400],
                ccg_out[CCG:CCN].rearrange("(col i) -> i col", i=48))

            # ======== Phase D: attention coefficient matrices =============
            with (
                tc.tile_pool(name="ph_d", bufs=1) as ph_d,
                tc.tile_pool(name="ps_d", bufs=1, space="PSUM") as ps_d,
            ):
                # norms: sumsq in g_buf[:, 384:400] (48, 16):
                # col h = ||q_i||^2 head h, col 8+h = ||k_i||^2 head h
                sumsq = ph_d.tile([48, 16], F32, tag="sumsq")
                nc.vector.tensor_scalar_max(
                    sumsq[:], g_buf[:, 384:400], 1.0e-12)
                nrm = ph_d.tile([48, 16], F32, tag="nrm")
                nc.scalar.sqrt(nrm[:], sumsq[:])
                # prefetch the exp table set right after the norm sqrt (the
                # nrm input pins the schedule); the load hides behind the
                # DVE-side norm broadcast and the first sort chain
                nc.scalar.activation(scr1[0:1, 3:4], nrm[0:1, 0:1], ACTF.Exp)
                rns = ph_d.tile([48, 16], F32, tag="rns")
                nc.vector.reciprocal(rns[:], nrm[:])

                # transpose rns -> (16, 48): rows = q/k x head, cols = channel
                rtp = ps_d.tile([48, 48], F32, tag="rtp")
                nc.tensor.transpose(rtp[0:16, :], rns[:],
                                    identf[0:48, 0:48])
                rnsT = ph_d.tile([16, 48], F32, tag="rnsT")
                nc.scalar.copy(rnsT[:], rtp[0:16, 0:48])

                # broadcast k-norms along partitions: per half, mask-select
                # the 4 needed rows of rnsT into group-blocks and matmul with
                # an all-ones stationary (out[p, (g,c)] = rk[2g+half, c])
                rkb = ph_d.tile([96, 192], F32, tag="rkb")
                rk_stage = ph_d.tile([48, 192], F32, tag="rk_stage")
                for half in range(2):
                    rhs3 = ph_d.tile([16, 4, 48], F32, tag="rhs3",
                                     name=f"rhs3{half}")
                    nc.vector.tensor_tensor(
                        rhs3[:],
                        rnsT[:, None, :].to_broadcast([16, 4, 48]),
                        maskk[:, :, half, None].to_broadcast([16, 4, 48]),
                        op=ALU.mult)
                    rkps = ps_d.tile([48, 192], F32, tag="rkps",
                                     name=f"rkps{half}")
                    nc.tensor.matmul(
                        rkps, lhsT=ones16[:],
                        rhs=rhs3.rearrange("p g d -> p (g d)"),
                        start=True, stop=True)
                    nc.scalar.copy(rkb[0:48, :] if half == 0 else rk_stage[:],
                                   rkps[:])
                nc.sync.dma_start(rkb[48:96, :], rk_stage[:])
                # q-norms, partition-aligned: rqb (96, 4); temperature folds
                # into the q side (A = rq*temp * G * rk)
                rqb = ph_d.tile([96, 4], F32, tag="rqb")
                rns2 = rns.rearrange("p (g x) -> p g x", x=2)
                nc.sync.dma_start(rqb[0:48, :], rns2[:, 0:4, 0])
                nc.sync.dma_start(rqb[48:96, :], rns2[:, 0:4, 1])
                nc.vector.tensor_tensor(rqb[:], rqb[:], tempg[:], op=ALU.mult)

                # A packed (96, 4*48): group g = heads (2g | 2g+1), loaded
                # straight from the AllReduce output in DRAM (fp16), scaled
                # into fp32 in aw
                a_all = ph_d.tile([96, 192], F16, tag="a_all")
                aw = ph_d.tile([96, 192], F32, tag="aw")
                g_v = ccg_out[0:CCG].rearrange(
                    "(c g two d) -> c g two d", g=4, two=2, d=48)
                for half in range(2):
                    nc.sync.dma_start(
                        a_all[half * 48:half * 48 + 48, :]
                        .rearrange("c (g d) -> c g d", d=48),
                        g_v[:, :, half, :])
                a3 = a_all.rearrange("p (g c) -> p g c", c=48)
                aw3 = aw.rearrange("p (g c) -> p g c", c=48)
                nc.vector.tensor_tensor(
                    aw3, a3,
                    rqb[:, :, None].to_broadcast([96, 4, 48]),
                    op=ALU.mult)
                nc.vector.tensor_tensor(aw[:], aw[:], rkb[:], op=ALU.mult)

                # Per-group pipeline: top-40 selection -> coefficients ->
                # CW -> block-diag transpose -> CW @ v + gelu. Group g's
                # PE/ACT tail overlaps group g+1's DVE sort chain.
                srt = ph_d.tile([96, 4, 40], F32, tag="sorted")
                scr = ph_d.tile([96, 192], F32, tag="scratch")
                es = ph_d.tile([96, 4, 40], F32, tag="esort")
                nrow = ph_d.tile([96, 4], F32, tag="nrow")
                sall = ph_d.tile([96, 4, 4], F32, tag="sall")
                call = ph_d.tile([96, 4, 4], F32, tag="call")
                msum = ph_d.tile([96, 192], F32, tag="msum")
                mb_t = ph_d.tile([96, 192], F32, tag="mb")
                cw = ph_d.tile([96, 192], F32, tag="cw")
                cwh = ph_d.tile([96, 192], F16, tag="cwh")
                cwt_l = [ph_d.tile([96, 96], F16, tag=f"cwt{g}",
                                   name=f"cwt{g}") for g in range(4)]
                at4 = atile.rearrange("p (g b) -> p g b", b=4)

                with (
                    tc.tile_pool(name="ph_e", bufs=2) as ph_e,
                    tc.tile_pool(name="ps_e", bufs=2, space="PSUM") as ps_e,
                    tc.tile_pool(name="ps_w", bufs=1, space="PSUM") as ps_w,
                ):
                    def pe_warm(src_ap, nm):
                        # dependency-spread dummy work: keeps the PE's HAM
                        # activity window open through the DVE-serial stretch
                        # so the tail matmuls run at full clock. The first
                        # transpose anchors the timing to the D-chain; the
                        # matmuls add enough duty cycle to register as busy.
                        warm = ps_w.tile([48, 512], F32, tag="warm", name=nm)
                        nc.tensor.transpose(warm[:, 0:96], src_ap, identf[:])
                        for ww in range(2):
                            nc.tensor.matmul(
                                warm[:, 0:512], lhsT=ident[:, 0:48],
                                rhs=x_sb[0][:, 0:512],
                                start=True, stop=True)

                    for g in range(4):
                        src = aw[:, g * 48:(g + 1) * 48]
                        dst = scr[:, g * 48:(g + 1) * 48]
                        # top-40 per row via 5 rounds of max8 + match_replace
                        # (the final round needs no replace)
                        for r in range(5):
                            nc.vector.max(srt[:, g, r * 8:(r + 1) * 8],
                                          src if r == 0 else dst)
                            if r < 4:
                                nc.vector.match_replace(
                                    out=dst,
                                    in_to_replace=srt[:, g, r * 8:(r + 1) * 8],
                                    in_values=src if r == 0 else dst,
                                    imm_value=NEG)
                        # prefix sums of exp(sorted - rowmax): fused exp +
                        # free-dim accumulate on ACT (bias = -rowmax)
                        nc.scalar.mul(nrow[:, g:g + 1], srt[:, g, 0:1], -1.0)
                        nc.scalar.activation(
                            es[:, g, 0:KKS[0]], srt[:, g, 0:KKS[0]],
                            ACTF.Exp, bias=nrow[:, g:g + 1], scale=1.0,
                            accum_out=sall[:, g, 0:1])
                        for bb in range(1, 4):
                            nc.scalar.activation(
                                es[:, g, KKS[bb - 1]:KKS[bb]],
                                srt[:, g, KKS[bb - 1]:KKS[bb]],
                                ACTF.Exp, bias=nrow[:, g:g + 1], scale=1.0,
                                accum_out=sall[:, g, bb:bb + 1])
                            nc.vector.tensor_add(
                                sall[:, g, bb:bb + 1], sall[:, g, bb:bb + 1],
                                sall[:, g, bb - 1:bb])
                        nc.vector.reciprocal(call[:, g, :], sall[:, g, :])
                        nc.vector.tensor_tensor(
                            call[:, g, :], call[:, g, :], at4[:, g, :],
                            op=ALU.mult)
                        pe_warm(scr[:, g * 48:(g + 1) * 48], f"warm_a{g}")
                        # msum = sum_b c_b*[A >= t_b]; CW = exp(A-rowmax)*msum
                        # fused per branch: (A is_ge t_b) * c_b in one op
                        for bb in range(4):
                            tgt = (msum if bb == 0 else mb_t)[:, g * 48:
                                                              (g + 1) * 48]
                            nc.vector.tensor_scalar(
                                tgt, src,
                                srt[:, g, KKS[bb] - 1:KKS[bb]],
                                call[:, g, bb:bb + 1],
                                op0=ALU.is_ge, op1=ALU.mult)
                            if bb > 0:
                                nc.vector.tensor_add(
                                    msum[:, g * 48:(g + 1) * 48],
                                    msum[:, g * 48:(g + 1) * 48], tgt)
                        pe_warm(msum[:, g * 48:(g + 1) * 48], f"warm_b{g}")
                        cwg = cw[:, g * 48:(g + 1) * 48]
                        nc.scalar.activation(
                            cwg, src, ACTF.Exp, bias=nrow[:, g:g + 1],
                            scale=1.0)
                        nc.vector.tensor_tensor(
                            cwh[:, g * 48:(g + 1) * 48], cwg,
                            msum[:, g * 48:(g + 1) * 48], op=ALU.mult)
                        # block-diag CW -> transpose -> cwt; CW @ v + gelu
                        # immediately so the PE fills during the next
                        # group's sort chain
                        bd = ph_e.tile([96, 96], F16, tag="bdiag")
                        nc.vector.memset(bd[:], 0.0)
                        nc.vector.tensor_copy(
                            bd[0:48, 0:48], cwh[0:48, g * 48:(g + 1) * 48])
                        nc.sync.dma_start(
                            bd[48:96, 48:96], cwh[48:96, g * 48:(g + 1) * 48])
                        tps = ps_d.tile([96, 96], F16, tag="tps")
                        nc.tensor.transpose(tps[:], bd[:], ident[0:96, 0:96])
                        nc.vector.tensor_copy(cwt_l[g][:], tps[:])
                        for s in range(3):
                            ops = ps_e.tile([96, 384], F32, tag="ops",
                                            name=f"ops{g}_{s}")
                            nc.tensor.matmul(
                                ops, lhsT=cwt_l[g][:],
                                rhs=v_pair[g][:, s * 384:(s + 1) * 384],
                                start=True, stop=True)
                            nc.scalar.activation(
                                y_pair[g][:, s * 384:(s + 1) * 384], ops[:],
                                ACTF.Gelu)

                # ==== Phase F: 1x1 proj at pooled res =====================
                with (
                    tc.tile_pool(name="ps_f", bufs=3, space="PSUM") as ps_f,
                    tc.tile_pool(name="ph_f", bufs=2) as ph_f,
                ):
                    for m in range(3):
                        pj = ph_f.tile([128, PRC * WP], F16, tag="pj",
                                       name=f"pj{m}")
                        for s in range(3):
                            psf = ps_f.tile([128, 384], F32, tag="ps_f",
                                            name=f"psf{s}_{m}")
                            for k in range(4):
                                nc.tensor.matmul(
                                    psf,
                                    lhsT=wproj[:, k, m * 128:(m + 1) * 128],
                                    rhs=y_pair[k][:, s * 384:(s + 1) * 384],
                                    start=(k == 0),
                                    stop=(k == 3),
                                )
                            # bias evict on DVE: ACT is busy with the gelus
                            nc.vector.tensor_scalar(
                                pj[:, s * 384:(s + 1) * 384], psf[:],
                                bproj[:, m:m + 1], None, op0=ALU.add)
                        nc.sync.dma_start(y_out[m, :, :], pj[:])

    _split_sync_waits(nc)
    return nc


# ----------------------------------------------------------------------------
# Host-side input preparation / sharding / gather
# ----------------------------------------------------------------------------

def _prep_core_inputs(x, w_qkv, b_qkv, w_dw, b_dw, w_proj, b_proj,
                      temperature, a1, a2, a3, a4):
    x = np.asarray(x, np.float32).reshape(DIM, HF, WF)
    w_qkv = np.asarray(w_qkv, np.float32)
    w_dw = np.asarray(w_dw, np.float32).reshape(3 * DIM, 3, 3)
    w_proj = np.asarray(w_proj, np.float32)

    wqkvT = np.ascontiguousarray(w_qkv.T).reshape(3, 128, 3 * DIM).astype(np.float16)
    wprojT = np.ascontiguousarray(w_proj.T).reshape(4, 96, DIM).astype(np.float16)

    # packed depthwise weights: dwp[p, m, h, r, c] applies tap PAIRS[r][p//64]
    # to channel m*128+h*64+c (diagonal in c = p%64); dw8 is the tap-8 diag.
    dwp = np.zeros((128, MB, 2, 4, 64), np.float32)
    pa = np.arange(128)
    ca = pa % 64
    for m in range(MB):
        for h in range(2):
            for r, (t0, t1) in enumerate(PAIRS):
                taps = np.where(pa < 64, t0, t1)
                dy = np.array([TAPS[t][0] for t in taps]) + 1
                dx = np.array([TAPS[t][1] for t in taps]) + 1
                dwp[pa, m, h, r, ca] = w_dw[m * 128 + h * 64 + ca, dy, dx]
    dw8 = np.zeros((128, MB, 128), np.float32)
    for m in range(MB):
        dw8[np.arange(128), m, np.arange(128)] = w_dw[m * 128 + np.arange(128), 2, 2]

    bq = np.asarray(b_qkv, np.float32).reshape(MB, 128)
    bd = np.asarray(b_dw, np.float32).reshape(MB, 128)
    bp = np.asarray(b_proj, np.float32).reshape(3, 128)

    ident = np.eye(128, dtype=np.float16)
    t8 = np.asarray(temperature, np.float32).reshape(HEADS)
    tempg = np.empty((96, 4), np.float32)
    for g in range(4):
        tempg[0:48, g] = t8[2 * g]
        tempg[48:96, g] = t8[2 * g + 1]
    maskk_h = np.zeros((16, 4, 2), np.float32)
    for g in range(4):
        for half in range(2):
            maskk_h[8 + 2 * g + half, g, half] = 1.0
    maskk_h = maskk_h.reshape(16, 8)
    avec = np.array([np.float32(a1[0]), np.float32(a2[0]),
                     np.float32(a3[0]), np.float32(a4[0])], np.float32)
    atile = np.tile(avec, (96, 4)).astype(np.float32)

    # x: pad 2 halo rows of zeros top/bottom, slice per core, cast fp16
    xp = np.zeros((DIM, HF + 4, WF), np.float16)
    xp[:, 2:HF + 2, :] = x
    in_maps = []
    for c in range(NC):
        xs = xp[:, c * RPC:c * RPC + XR, :]                  # (384, 28, 192)
        xs = xs.reshape(3, 128, XR * WF).reshape(3, 128, 4, 1344)
        xs = np.ascontiguousarray(xs.transpose(0, 2, 1, 3))
        bqkv3 = np.stack([
            bq.T, bq.T * (1.0 if c > 0 else 0.0),
            bq.T * (1.0 if c < NC - 1 else 0.0)], axis=2)     # (128, 9, 3)
        in_maps.append({
            "x_slice": xs,
            "wqkvT": wqkvT,
            "dwp": dwp.astype(np.float16),
            "dw8": dw8.astype(np.float16),
            "wprojT": wprojT,
            "bqkv": np.ascontiguousarray(bqkv3, np.float32),
            "bdw": np.ascontiguousarray(bd.T),
            "bproj": np.ascontiguousarray(bp.T),
            "ident": ident,
            "tempg": tempg,
            "maskk": maskk_h,
            "atile": atile,
        })
    return in_maps


_CACHE = {}
_CACHE_LOCK = threading.Lock()


def _make_runner():
    """Compile once; return a callable in_maps -> list[{name: array}].

    Mirrors concourse.bass2jax.run_bass_via_pjrt but caches the jitted
    executable so repeat kernel() calls do not recompile.
    """
    import jax
    import concourse.mybir as mybir
    from concourse import bass2jax
    from jax.experimental.shard_map import shard_map
    from jax.sharding import Mesh, PartitionSpec

    nc = build_kernel()
    bass2jax.install_neuronx_cc_hook()
    partition_name = (nc.partition_id_tensor.name
                      if nc.partition_id_tensor else None)
    in_names, out_names, out_avals, zero_outs = [], [], [], []
    for alloc in nc.m.functions[0].allocations:
        if not isinstance(alloc, mybir.MemoryLocationSet):
            continue
        name = alloc.memorylocations[0].name
        if alloc.kind == "ExternalInput":
            if name != partition_name:
                in_names.append(name)
        elif alloc.kind == "ExternalOutput":
            shape = tuple(alloc.tensor_shape)
            dtype = mybir.dt.np(alloc.dtype)
            out_names.append(name)
            out_avals.append(jax.core.ShapedArray(shape, dtype))
            zero_outs.append(np.zeros(shape, dtype))
    n_params = len(in_names)
    n_outs = len(out_avals)
    all_names = list(in_names) + list(out_names)
    if partition_name is not None:
        all_names.append(partition_name)
    donate = tuple(range(n_params, n_params + n_outs))

    def _body(*args):
        operands = list(args)
        if partition_name is not None:
            operands.append(bass2jax.partition_id_tensor())
        return tuple(bass2jax._bass_exec_p.bind(
            *operands,
            out_avals=tuple(out_avals),
            in_names=tuple(all_names),
            out_names=tuple(out_names),
            lowering_input_output_aliases=(),
            sim_require_finite=True,
            sim_require_nnan=True,
            nc=nc,
        ))

    devices = jax.devices()[:NC]
    mesh = Mesh(np.asarray(devices), ("core",))
    in_specs = (PartitionSpec("core"),) * (n_params + n_outs)
    out_specs = (PartitionSpec("core"),) * n_outs
    sharded = jax.jit(
        shard_map(_body, mesh=mesh, in_specs=in_specs, out_specs=out_specs,
                  check_rep=False),
        donate_argnums=donate, keep_unused=True)

    import jax.numpy as jnp
    sharding = jax.sharding.NamedSharding(mesh, PartitionSpec("core"))
    zeros_dev = jax.jit(
        lambda: tuple(
            jnp.zeros((NC * z.shape[0], *z.shape[1:]), z.dtype)
            for z in zero_outs),
        out_shardings=tuple(sharding for _ in zero_outs))

    def upload(in_maps):
        concat_in = [
            np.concatenate([np.asarray(in_maps[c][nm]) for c in range(NC)],
                           axis=0)
            for nm in in_names[:n_params]
        ]
        return [jax.device_put(a, sharding) for a in concat_in]

    def execute(dev_args):
        out_arrs = sharded(*dev_args, *zeros_dev())
        jax.block_until_ready(out_arrs)
        return out_arrs

    def run(in_maps):
        out_arrs = execute(upload(in_maps))
        return [
            {nm: np.asarray(out_arrs[i]).reshape(NC, *out_avals[i].shape)[c]
             for i, nm in enumerate(out_names)}
            for c in range(NC)
        ]

    run.upload = upload
    run.execute = execute
    return run


def kernel(**inputs) -> np.ndarray:
    with _CACHE_LOCK:
        runner = _CACHE.get("runner")
        if runner is None:
            runner = _make_runner()
            _CACHE["runner"] = runner
    in_maps = _prep_core_inputs(**inputs)
    results = runner(in_maps)
    out = np.empty((1, DIM, HF, WF), np.float32)
    for c in range(NC):
        y = results[c]["y_slice"].astype(np.float32).reshape(DIM, PRC, WP)
        up = np.repeat(np.repeat(y, DS, axis=1), DS, axis=2)
        out[0, :, c * RPC:(c + 1) * RPC, :] = up
    return out
